# revision 7
# baseline (speedup 1.0000x reference)
"""Trainium2 Bass kernel for BioDetailedSTDPSNN.

Strategy (8-way batch-parallel, B_local=32 per core):
  - pre = x@W_in + b_in batched per 16-step chunk on PE (3-pass bf16 hi/lo split,
    ~2^-17 accuracy; X transposed/split on host).
  - Serial LIF scan with A-substitution (A = V + k, k = decayed cth accumulator):
    per step: A' = 0.9A + pre_t; s = A' > krow (threshold tensor built by a K=1
    PE outer product); A = A' - s.  Per-batch scalar chain (u = cth + 0.1, krow
    = 1 + k) runs on tiny [1,32]/[1,128] DVE ops; rate via a ones-matmul.
  - postin = s@W_post, logits = s@W_head batched per chunk (bf16).
  - stdp_signal: G[tau,sig] = sum_{b,h} pre_tau*postin_sig Gram matrix computed
    on PE from fp8 copies; host applies the EMA-weight sandwich to get stdp_t.
  - th_seq/sr_seq recovered on host from the exact u-trajectory (fp32, DMA'd out).
Outputs are assembled on host to match the reference 5-tuple.
"""

import numpy as np
import ml_dtypes

T, B, DIN, H, DOUT = 512, 256, 512, 512, 128
NCORES = 8
BL = B // NCORES          # 32
TC = 16                   # chunk length
NCH = T // TC             # 32 chunks
GAMMA = 0.02 / H          # rate-sum -> cth scale
DELTA = 0.003             # u-recurrence constant (u = cth + 0.1)

bf16 = ml_dtypes.bfloat16
f8 = ml_dtypes.float8_e4m3

_CACHE = {}


def _build_nc():
    import concourse.bacc as bacc
    import concourse.mybir as mybir
    from concourse.tile import TileContext

    F32 = mybir.dt.float32
    BF = mybir.dt.bfloat16
    F8 = mybir.dt.float8e4
    ALU = mybir.AluOpType
    AXF = mybir.ActivationFunctionType
    AX = mybir.AxisListType

    nc = bacc.Bacc(None, target_bir_lowering=False)

    # ---- DRAM I/O ----
    xhiT = nc.dram_tensor("xhiT", [DIN, T * BL], BF, kind="ExternalInput")
    xloT = nc.dram_tensor("xloT", [DIN, T * BL], BF, kind="ExternalInput")
    winh = nc.dram_tensor("winh", [DIN, H], BF, kind="ExternalInput")
    winl = nc.dram_tensor("winl", [DIN, H], BF, kind="ExternalInput")
    wpost = nc.dram_tensor("wpost", [H, H], BF, kind="ExternalInput")
    whead = nc.dram_tensor("whead", [H, DOUT], BF, kind="ExternalInput")
    bin_d = nc.dram_tensor("bin", [H, 1], F32, kind="ExternalInput")
    bpost_d = nc.dram_tensor("bpost", [H, 1], F32, kind="ExternalInput")
    bhead_d = nc.dram_tensor("bhead", [DOUT, 1], F32, kind="ExternalInput")

    lgt = nc.dram_tensor("lgt", [DOUT, T * BL], F32, kind="ExternalOutput")
    g_out = nc.dram_tensor("g", [T, T], F32, kind="ExternalOutput")
    u_out = nc.dram_tensor("uo", [T, BL], F32, kind="ExternalOutput")

    with TileContext(nc) as tc:
        with tc.tile_pool(name="w", bufs=1) as wp, \
             tc.tile_pool(name="state", bufs=1) as sp, \
             tc.tile_pool(name="big", bufs=1) as bigp, \
             tc.tile_pool(name="xt", bufs=2) as xp, \
             tc.tile_pool(name="chunk", bufs=2) as cp, \
             tc.tile_pool(name="out", bufs=2) as op:

            # ---- persistent weights ----
            winh_sb = []
            winl_sb = []
            wpost_sb = []
            whead_sb = []
            for k in range(4):
                th = wp.tile([128, H], BF, tag=f"winh{k}")
                nc.sync.dma_start(th[:], winh[k * 128:(k + 1) * 128, :])
                winh_sb.append(th)
                tl = wp.tile([128, H], BF, tag=f"winl{k}")
                nc.sync.dma_start(tl[:], winl[k * 128:(k + 1) * 128, :])
                winl_sb.append(tl)
                tp = wp.tile([128, H], BF, tag=f"wpost{k}")
                nc.sync.dma_start(tp[:], wpost[k * 128:(k + 1) * 128, :])
                wpost_sb.append(tp)
                td = wp.tile([128, DOUT], BF, tag=f"whead{k}")
                nc.sync.dma_start(td[:], whead[k * 128:(k + 1) * 128, :])
                whead_sb.append(td)
            bin_sb = []
            bpost_sb = []
            for k in range(4):
                tb = wp.tile([128, 1], F32, tag=f"bin{k}")
                nc.sync.dma_start(tb[:], bin_d[k * 128:(k + 1) * 128, :])
                bin_sb.append(tb)
                tb2 = wp.tile([128, 1], F32, tag=f"bpost{k}")
                nc.sync.dma_start(tb2[:], bpost_d[k * 128:(k + 1) * 128, :])
                bpost_sb.append(tb2)
            bhead_sb = wp.tile([128, 1], F32, tag="bhead")
            nc.sync.dma_start(bhead_sb[:], bhead_d[:, :])

            ones1f = wp.tile([1, 128], F32, tag="ones1f")
            nc.vector.memset(ones1f[:], 1.0)
            onesKb = wp.tile([128, 1], BF, tag="onesKb")
            nc.vector.memset(onesKb[:], 1.0)

            # ---- persistent state ----
            A = sp.tile([128, 128], F32, tag="A")       # [h', (m,b)]
            Ap = sp.tile([128, 128], F32, tag="Ap")
            krow = sp.tile([1, 128], F32, tag="krow")   # 1 + k, free=(m,b)
            ubuf = sp.tile([1, 2 * TC * 32], F32, tag="ubuf")
            r32 = sp.tile([1, 32], F32, tag="r32")
            rp = sp.tile([1, 32], F32, tag="rp")
            nc.vector.memset(A[:], 0.0)
            nc.vector.memset(krow[:], 1.0)
            nc.vector.memset(ubuf[:], 0.1)

            # ---- persistent fp8 gram operands ----
            pre8 = bigp.tile([128, T * 128], F8, tag="pre8")    # [h', (t,m,b)]
            post8 = bigp.tile([128, T * 128], F8, tag="post8")  # [j', (t,m,b)]

            with tc.tile_pool(name="ppre", bufs=2, space="PSUM") as ppre, \
                 tc.tile_pool(name="ppost", bufs=2, space="PSUM") as ppost, \
                 tc.tile_pool(name="plog", bufs=1, space="PSUM") as plog, \
                 tc.tile_pool(name="pkf", bufs=2, space="PSUM") as pkf, \
                 tc.tile_pool(name="pr", bufs=1, space="PSUM") as prp:

                for c in range(NCH):
                    cs = c * TC * BL          # column offset in (t,b) space
                    # -- load transposed x slices --
                    xth = []
                    xtl = []
                    for k in range(4):
                        a = xp.tile([128, TC * BL], BF, tag=f"xth{k}")
                        nc.sync.dma_start(a[:], xhiT[k * 128:(k + 1) * 128,
                                                     cs:cs + TC * BL])
                        xth.append(a)
                        b = xp.tile([128, TC * BL], BF, tag=f"xtl{k}")
                        nc.sync.dma_start(b[:], xloT[k * 128:(k + 1) * 128,
                                                     cs:cs + TC * BL])
                        xtl.append(b)

                    pre_f32 = cp.tile([128, TC * 128], F32, tag="pre_f32")
                    s_chunk = cp.tile([128, TC * 128], BF, tag="s_chunk")

                    # -- pre matmuls: 3-pass bf16 --
                    for m in range(4):
                        ps = ppre.tile([128, TC * BL], F32, tag="ppre")
                        i = 0
                        for wt, xt_ in ((winh_sb, xth), (winh_sb, xtl),
                                        (winl_sb, xth)):
                            for k in range(4):
                                nc.tensor.matmul(
                                    ps[:], wt[k][:, m * 128:(m + 1) * 128],
                                    xt_[k][:], start=(i == 0), stop=(i == 11))
                                i += 1
                        # psum [h' , (t,b)] -> pre_f32 [h', t, m, b] and fp8 copy
                        psv = ps[:].rearrange("p (t b) -> p t b", t=TC)
                        dst32 = pre_f32[:].rearrange(
                            "p (t m b) -> p t m b", t=TC, m=4)[:, :, m, :]
                        nc.scalar.activation(dst32, psv, AXF.Identity,
                                             bias=bin_sb[m][:])
                        dst8 = pre8[:].rearrange(
                            "p (t m b) -> p t m b", t=T, m=4)[:, c * TC:(c + 1) * TC, m, :]
                        nc.scalar.activation(dst8, psv, AXF.Identity,
                                             bias=bin_sb[m][:])

                    # -- serial LIF steps --
                    pre_v = pre_f32[:].rearrange("p (t f) -> p t f", t=TC)
                    s_v = s_chunk[:].rearrange("p (t f) -> p t f", t=TC)
                    for t in range(TC):
                        gt = c * TC + t
                        cur = ((c % 2) * TC + t) * 32
                        prv = (((c + 1) % 2) * TC + TC - 1) * 32 if t == 0 \
                            else ((c % 2) * TC + t - 1) * 32
                        # A' = 0.9A + pre_t
                        nc.vector.scalar_tensor_tensor(
                            Ap[:], A[:], 0.9, pre_v[:, t], op0=ALU.mult,
                            op1=ALU.add)
                        # threshold tensor: kfull = ones^T krow (outer, fp32)
                        kps = pkf.tile([128, 128], F32, tag="kf")
                        nc.tensor.matmul(kps[:], ones1f[:], krow[:],
                                         start=True, stop=True)
                        # s = A' > kfull  (bf16)
                        nc.vector.tensor_tensor(s_v[:, t], Ap[:], kps[:],
                                                op=ALU.is_gt)
                        # rate column sums: [1, (m,b)]
                        prt = prp.tile([1, 128], F32, tag="pr")
                        nc.tensor.matmul(prt[:], onesKb[:], s_v[:, t],
                                         start=True, stop=True)
                        # A = A' - s
                        nc.vector.scalar_tensor_tensor(
                            A[:], Ap[:], 0.0, s_v[:, t], op0=ALU.add,
                            op1=ALU.subtract)
                        # r32[b] = sum_m prt[(m,b)]
                        nc.vector.tensor_reduce(
                            r32[:], prt[:].rearrange("p (m b) -> p b m", m=4),
                            axis=AX.X, op=ALU.add)
                        # rp = GAMMA*r32 + DELTA
                        nc.vector.tensor_scalar(rp[:], r32[:], GAMMA, DELTA,
                                                op0=ALU.mult, op1=ALU.add)
                        # u_t = 0.95*u_{t-1} + rp
                        nc.vector.scalar_tensor_tensor(
                            ubuf[:, cur:cur + 32], ubuf[:, prv:prv + 32], 0.95,
                            rp[:], op0=ALU.mult, op1=ALU.add)
                        # krow = 0.9*krow + bcast_m(u_t)
                        ub = ubuf[:, cur:cur + 32].unsqueeze(1) \
                            .to_broadcast([1, 4, 32])
                        nc.vector.scalar_tensor_tensor(
                            krow[:].rearrange("p (m b) -> p m b", m=4),
                            krow[:].rearrange("p (m b) -> p m b", m=4), 0.9,
                            ub, op0=ALU.mult, op1=ALU.add)

                    # -- postin matmuls (fp8 out for gram) --
                    for jm in range(4):
                        ps2 = ppost.tile([128, TC * BL], F32, tag="ppost")
                        for k in range(4):
                            rhs = s_chunk[:].rearrange(
                                "p (t m b) -> p t m b", t=TC, m=4)[:, :, k, :]
                            nc.tensor.matmul(
                                ps2[:].rearrange("p (t b) -> p t b", t=TC),
                                wpost_sb[k][:, jm * 128:(jm + 1) * 128], rhs,
                                start=(k == 0), stop=(k == 3))
                        dstp = post8[:].rearrange(
                            "p (t m b) -> p t m b", t=T, m=4)[:, c * TC:(c + 1) * TC, jm, :]
                        nc.scalar.activation(
                            dstp, ps2[:].rearrange("p (t b) -> p t b", t=TC),
                            AXF.Identity, bias=bpost_sb[jm][:])

                    # -- logits matmuls --
                    pl = plog.tile([128, TC * BL], F32, tag="plog")
                    for k in range(4):
                        rhs = s_chunk[:].rearrange(
                            "p (t m b) -> p t m b", t=TC, m=4)[:, :, k, :]
                        nc.tensor.matmul(
                            pl[:].rearrange("p (t b) -> p t b", t=TC),
                            whead_sb[k][:], rhs, start=(k == 0), stop=(k == 3))
                    lg_sb = op.tile([128, TC * BL], F32, tag="lg")
                    nc.scalar.activation(lg_sb[:], pl[:], AXF.Identity,
                                         bias=bhead_sb[:])
                    nc.sync.dma_start(lgt[:, cs:cs + TC * BL], lg_sb[:])

                    # -- u trajectory dump --
                    nc.sync.dma_start(
                        u_out[c * TC:(c + 1) * TC, :]
                        .rearrange("a b -> (a b)").unsqueeze(0),
                        ubuf[:, (c % 2) * TC * 32:((c % 2) + 1) * TC * 32])

            # -- Gram: G[tau, sig] = sum_{m,b,h'} pre8 * post8 --
            if True:
                with tc.tile_pool(name="pg", bufs=2, space="PSUM") as pg:
                    pre8v = pre8[:].rearrange("p (t m b) -> p t m b", t=T, m=4)
                    post8v = post8[:].rearrange("p (t m b) -> p t m b", t=T, m=4)
                    mg = min(128, T)
                    for tg in range(T // mg):
                        gps = pg.tile([mg, T], F32, tag="gps")
                        i = 0
                        for m in range(4):
                            for b in range(32):
                                nc.tensor.matmul(
                                    gps[:],
                                    pre8v[:, tg * mg:(tg + 1) * mg, m, b],
                                    post8v[:, :, m, b],
                                    start=(i == 0), stop=(i == 127))
                                i += 1
                        gsb = op.tile([mg, T], F32, tag="gsb")
                        nc.scalar.activation(gsb[:], gps[:], AXF.Identity)
                        nc.sync.dma_start(g_out[tg * mg:(tg + 1) * mg, :],
                                          gsb[:])

    nc.finalize()
    return nc


def _get_nc():
    if "nc" not in _CACHE:
        _CACHE["nc"] = _build_nc()
    return _CACHE["nc"]


def kernel(x_seq, W_in, b_in, W_post, b_post, W_head, b_head):
    from concourse.bass_utils import run_bass_kernel_spmd

    x_seq = np.asarray(x_seq, np.float32)
    W_in = np.asarray(W_in, np.float32)
    b_in = np.asarray(b_in, np.float32)
    W_post = np.asarray(W_post, np.float32)
    b_post = np.asarray(b_post, np.float32)
    W_head = np.asarray(W_head, np.float32)
    b_head = np.asarray(b_head, np.float32)

    winh = W_in.astype(bf16)
    winl = (W_in - winh.astype(np.float32)).astype(bf16)
    wpost_b = W_post.astype(bf16)
    whead_b = W_head.astype(bf16)
    shared = dict(
        winh=winh, winl=winl, wpost=wpost_b, whead=whead_b,
        bin=b_in.reshape(H, 1).astype(np.float32),
        bpost=b_post.reshape(H, 1).astype(np.float32),
        bhead=b_head.reshape(DOUT, 1).astype(np.float32),
    )
    in_maps = []
    for c in range(NCORES):
        xs = x_seq[:, c * BL:(c + 1) * BL, :]          # [T, BL, DIN]
        xt = np.ascontiguousarray(xs.transpose(2, 0, 1)).reshape(DIN, T * BL)
        xhiT = xt.astype(bf16)
        xloT = (xt - xhiT.astype(np.float32)).astype(bf16)
        in_maps.append(dict(shared, xhiT=xhiT, xloT=xloT))

    nc = _get_nc()
    res = run_bass_kernel_spmd(nc, in_maps, core_ids=list(range(NCORES)),
                               **_CACHE.get("run_kwargs", {}))
    _CACHE["last_res"] = res
    results = res.results

    # ---- host assembly ----
    logits_seq = np.zeros((T, B, DOUT), np.float32)
    G = np.zeros((T, T), np.float64)
    Rsum_t = np.zeros(T, np.float64)       # total spike count per step
    cth_sum_t = np.zeros(T, np.float64)    # sum_b cth per step
    for c in range(NCORES):
        r = results[c]
        lg = r["lgt"].reshape(DOUT, T, BL).transpose(1, 2, 0)
        logits_seq[:, c * BL:(c + 1) * BL, :] = lg
        G += r["g"].astype(np.float64)
        u = r["uo"].astype(np.float64)                     # [T, BL]
        uprev = np.vstack([np.full((1, BL), 0.1), u[:-1]])
        Rsum_t += ((u - 0.95 * uprev - DELTA) / GAMMA).sum(axis=1)
        cth_sum_t += (u - 0.1).sum(axis=1)

    # stdp_t = a_t^T G a_t / (B*H), a_t = 0.95 a_{t-1} + 0.05 e_t
    Gs = 0.5 * (G + G.T)
    stdp_seq = np.zeros(T, np.float64)
    mvec = np.zeros(T, np.float64)
    prev = 0.0
    for t in range(T):
        stdp_seq[t] = 0.9025 * prev + 0.1 * 0.95 * mvec[t] + 0.0025 * G[t, t]
        mvec = 0.95 * mvec + 0.05 * Gs[:, t]
        prev = stdp_seq[t]
    stdp_seq /= (B * H)

    logits_seq += (0.05 * stdp_seq)[:, None, None].astype(np.float32)
    out = logits_seq.mean(axis=0)
    sr_mean = Rsum_t.sum() / (T * B * H)
    th_mean = cth_sum_t.sum() / (T * B)
    return (out.astype(np.float32),
            logits_seq.astype(np.float32),
            np.array([sr_mean], np.float32),
            np.float32(stdp_seq.mean()),
            np.float32(th_mean))
